# revision 55
# baseline (speedup 1.0000x reference)
"""Trainium2 Bass kernel for nn_Attention_73203422593618 (sparse_attention).

Two-modality MQA attention layer (B=4, S0=768@W0=2048, S1=256@W1=1024,
T=1024, N=8 query heads, K=1 kv head, H=256, RoPE, causal-masked softmax),
returning (out0, out1, idx, k, v).

Sharding (8 cores): core c -> batch b=c//2 (data parallel), head group
g=c%2 (tensor parallel over 4 of the 8 query heads; MQA k/v replicated
within the pair).  Each core computes partial out0/out1 over its 4 heads;
the pair's two partials are summed on the host at unshard time (the TP
all-reduce).  k/v are produced identically by both cores of a pair; the
host takes the even core's copy.

Device dataflow (per core, all matmuls bf16 with fp32 PSUM accumulation):
  - activations/weights arrive host-swizzled into SBUF layout
    [128-partition, block, free] so every DMA is linear
  - v, k_nat in natural [s, i] layout (stationary = x^T tile), k_nat gets
    RoPE via transposed tables -> k/v DRAM outputs
  - q^T, k^T in [j, t] layout (stationary = weight tile), RoPE via [j, pos]
    tables (q scale H^-0.5 folded into q weights on host)
  - logits^T[s,t] = k^T.T @ q^T per s-tile; probs = exp(logits)*maskT
    (no max subtraction: logits are O(+-6) for this problem's data)
  - denom[1,t] via ones-matmul over s, reciprocal, PE outer-product
    broadcast to [128,t]; normalization folded into enc PSUM->SBUF copy
  - enc^T[i,t] = v.T @ probs^T; out0/out1 = enc_norm^T.T @ ow
"""

import sys

try:
    import concourse  # noqa: F401
except ImportError:  # pragma: no cover
    sys.path.insert(0, "/opt/trn_rl_repo")

import numpy as np
import ml_dtypes

import concourse.bass as bass
import concourse.bacc as bacc
import concourse.mybir as mybir
import concourse.tile as tile
from concourse.bass_utils import run_bass_kernel_spmd

F32 = mybir.dt.float32
BF16 = mybir.dt.bfloat16
MUL = mybir.AluOpType.mult
ADD = mybir.AluOpType.add
SUB = mybir.AluOpType.subtract
EXP = mybir.ActivationFunctionType.Exp
LN = mybir.ActivationFunctionType.Ln
COPY = mybir.ActivationFunctionType.Copy

B, S0, S1 = 4, 768, 256
T = S0 + S1          # 1024
W0, W1 = 2048, 1024
N, H = 8, 256
NH = N // 2          # heads per core (TP=2)
HH = H // 2          # rope half = 128
P = 128
MAX_WAVELENGTH = 10000.0

# position chunks (start, width); starts < S0 are modality 0
CHUNKS = [(0, 512), (512, 256), (768, 256)]
ND0, ND1 = W0 // P, W1 // P    # d-tiles per modality: 16, 8
NS = T // P                    # s-tiles: 8
NT0 = S0 // P                  # t-tiles in modality 0: 6
NT1 = S1 // P                  # t-tiles in modality 1: 2
NTC = 4                        # t-chunks for attention tiles
TCW = T // NTC                 # 256


def _x_slice(xt0, xt1, start, width):
    """Moving-operand source for x^T covering positions [start, start+width)."""
    if start < S0:
        assert start + width <= S0
        return xt0, ND0, start
    return xt1, ND1, start - S0


def build_nc(tile_status=None):
    if tile_status is None:
        tile_status = {(st, tc): 1 for st in range(NS) for tc in range(NTC)}
    live_tc = {st: [tc for tc in range(NTC) if tile_status[(st, tc)]]
               for st in range(NS)}
    pcol = {st: {tc: TCW * i for i, tc in enumerate(live_tc[st])} for st in range(NS)}
    active_st = {tc: [st for st in range(NS) if tile_status[(st, tc)]]
                 for tc in range(NTC)}
    # probs tile widths are quantized to {256, 512, 1024} and share two tags
    def _probw(st):
        n = len(live_tc[st])
        return 512 if n <= 2 else 1024
    n_full = sum(1 for st in range(NS) if _probw(st) == 1024)
    n_half = sum(1 for st in range(NS) if 0 < len(live_tc[st]) and _probw(st) == 512)
    # two heads in flight want 2x buffers; fall back to ~1x when SBUF is tight
    if 2 * (2 * n_full + n_half) > 28:
        bufs_full, bufs_half = n_full + 2, max(n_half + 2, 1)
        pipelined = False   # not enough probs buffers for two heads in flight
    else:
        bufs_full, bufs_half = max(2 * n_full, 1), max(2 * n_half, 1)
        pipelined = True

    nc = bacc.Bacc("TRN2", target_bir_lowering=False, debug=False, num_devices=8)

    xt0_d = nc.declare_dram_parameter("xt0", [P, ND0, S0], BF16, isOutput=False)
    xt1_d = nc.declare_dram_parameter("xt1", [P, ND1, S1], BF16, isOutput=False)
    qw0_d = nc.declare_dram_parameter("qw0", [NH, P, ND0, H], BF16, isOutput=False)
    qw1_d = nc.declare_dram_parameter("qw1", [NH, P, ND1, H], BF16, isOutput=False)
    kw0_d = nc.declare_dram_parameter("kw0", [P, ND0, H], BF16, isOutput=False)
    kw1_d = nc.declare_dram_parameter("kw1", [P, ND1, H], BF16, isOutput=False)
    vw0_d = nc.declare_dram_parameter("vw0", [P, ND0, H], BF16, isOutput=False)
    vw1_d = nc.declare_dram_parameter("vw1", [P, ND1, H], BF16, isOutput=False)
    ow0_d = nc.declare_dram_parameter("ow0", [P, NH * 2, W0], BF16, isOutput=False)
    ow1_d = nc.declare_dram_parameter("ow1", [P, NH * 2, W1], BF16, isOutput=False)
    mask_d = nc.declare_dram_parameter("maskT", [P, NS, T], BF16, isOutput=False)
    cos_d = nc.declare_dram_parameter("cosjt", [P, T], BF16, isOutput=False)
    sin_d = nc.declare_dram_parameter("sinjt", [P, T], BF16, isOutput=False)
    xk0_d = nc.declare_dram_parameter("xk0", [P, ND0, 3 * P], BF16, isOutput=False)
    xk1_d = nc.declare_dram_parameter("xk1", [P, ND1, P], BF16, isOutput=False)
    ckt_d = nc.declare_dram_parameter("ckt", [P, 4, HH], BF16, isOutput=False)
    skt_d = nc.declare_dram_parameter("skt", [P, 4, HH], BF16, isOutput=False)

    out0_d = nc.declare_dram_parameter("out0p", [S0, W0], F32, isOutput=True)
    out1_d = nc.declare_dram_parameter("out1p", [S1, W1], F32, isOutput=True)
    k_d = nc.declare_dram_parameter("k_out", [T // 2, H], F32, isOutput=True)
    v_d = nc.declare_dram_parameter("v_out", [T, H], F32, isOutput=True)

    with tile.TileContext(nc) as tc:
        with (
            tc.tile_pool(name="persist", bufs=1) as pp,
            tc.tile_pool(name="work", bufs=4) as tp,
            tc.tile_pool(name="probs", bufs=8) as probp,
            tc.tile_pool(name="small", bufs=2) as sp,
            tc.tile_pool(name="psum", bufs=7, space="PSUM") as psp,
            tc.tile_pool(name="psum_row", bufs=1, space="PSUM") as psrp,
        ):
            maskT = pp.tile([P, NS, T], BF16, tag="maskT")
            v_sb = pp.tile([P, NS, H], BF16, tag="v_sb")
            kT = pp.tile([P, 2, T], BF16, tag="kT")
            encn = pp.tile([P, NH * 2, T], BF16, tag="encn")
            ones_col = pp.tile([P, 1], BF16, tag="ones_col")
            ones_row = pp.tile([1, P], BF16, tag="ones_row")
            nc.vector.memset(ones_col[:], 1.0)
            nc.vector.memset(ones_row[:], 1.0)
            # HAM warm-up: dummy matmuls with no DMA dependency keep the PE
            # activity monitor busy while the first inputs stream in, so real
            # matmuls start at 2.4 GHz instead of 1.2 GHz
            warm = pp.tile([P, 512], BF16, tag="warm")
            nc.gpsimd.memset(warm[:], 0.0)
            pw = psp.tile([P, 512], F32, tag="mm")
            for i in range(12):
                nc.tensor.matmul(pw[:], warm[:, 0:P], warm[:],
                                 start=(i == 0), stop=(i == 11))

            with (
                tc.tile_pool(name="proj", bufs=1) as pj,
                tc.tile_pool(name="projw", bufs=2) as wp,
            ):
                xt0 = pj.tile([P, ND0, S0], BF16, tag="xt0")
                xt1 = pj.tile([P, ND1, S1], BF16, tag="xt1")
                cosjt = pj.tile([P, T], BF16, tag="cosjt")
                sinjt = pj.tile([P, T], BF16, tag="sinjt")
                xk0 = pj.tile([P, ND0, 3 * P], BF16, tag="xk0")
                xk1 = pj.tile([P, ND1, P], BF16, tag="xk1")
                ckt = pj.tile([P, 4, HH], BF16, tag="ckt")
                skt = pj.tile([P, 4, HH], BF16, tag="skt")

                def rope_jt(pj0, pj1, out_tile, c0, cw):
                    ta = tp.tile([P, 512], BF16, tag="rtmp2", bufs=6)
                    tb = tp.tile([P, 512], BF16, tag="rtmp2", bufs=6)
                    c_ = cosjt[:, c0:c0 + cw]
                    s_ = sinjt[:, c0:c0 + cw]
                    nc.vector.tensor_tensor(ta[:, :cw], pj0[:, :cw], c_, MUL)
                    nc.vector.tensor_tensor(tb[:, :cw], pj1[:, :cw], s_, MUL)
                    nc.vector.tensor_tensor(
                        out_tile[:, 0, c0:c0 + cw], ta[:, :cw], tb[:, :cw], SUB)
                    ta2 = tp.tile([P, 512], BF16, tag="rtmp2", bufs=6)
                    tb2 = tp.tile([P, 512], BF16, tag="rtmp2", bufs=6)
                    nc.vector.tensor_tensor(ta2[:, :cw], pj1[:, :cw], c_, MUL)
                    nc.vector.tensor_tensor(tb2[:, :cw], pj0[:, :cw], s_, MUL)
                    nc.vector.tensor_tensor(
                        out_tile[:, 1, c0:c0 + cw], ta2[:, :cw], tb2[:, :cw], ADD)

                def proj_jt(w0_sb, w1_sb, out_tile, chunks=None):
                    for c0, cw in (chunks or CHUNKS):
                        xsb, nd, off = _x_slice(xt0, xt1, c0, cw)
                        wsb = w0_sb if c0 < S0 else w1_sb
                        pj0 = psp.tile([P, 512], F32, tag="mm")
                        pj1 = psp.tile([P, 512], F32, tag="mm")
                        for jt, pjp in ((0, pj0), (1, pj1)):
                            for dt in range(nd):
                                nc.tensor.matmul(
                                    pjp[:, :cw], wsb[:, dt, jt * P:(jt + 1) * P],
                                    xsb[:, dt, off:off + cw],
                                    start=(dt == 0), stop=(dt == nd - 1),
                                )
                        rope_jt(pj0, pj1, out_tile, c0, cw)

                with tc.tile_pool(name="projkv", bufs=1) as pjw:
                    kw0 = pjw.tile([P, ND0, H], BF16, tag="kw0")
                    kw1 = pjw.tile([P, ND1, H], BF16, tag="kw1")
                    vw0 = pjw.tile([P, ND0, H], BF16, tag="vw0")
                    vw1 = pjw.tile([P, ND1, H], BF16, tag="vw1")

                    nc.sync.dma_start(xt1[:], xt1_d[:])
                    nc.sync.dma_start(vw1[:], vw1_d[:])
                    nc.sync.dma_start(kw1[:], kw1_d[:])
                    nc.sync.dma_start(xk1[:], xk1_d[:])
                    nc.sync.dma_start(ckt[:], ckt_d[:])
                    nc.sync.dma_start(skt[:], skt_d[:])
                    nc.sync.dma_start(vw0[:], vw0_d[:])
                    for dq in range(4):
                        nc.sync.dma_start(xt0[:, dq * 4:(dq + 1) * 4, :],
                                          xt0_d[:, dq * 4:(dq + 1) * 4, :])
                    nc.sync.dma_start(kw0[:], kw0_d[:])
                    nc.sync.dma_start(xk0[:], xk0_d[:])
                    nc.sync.dma_start(cosjt[:], cos_d[:])
                    nc.sync.dma_start(sinjt[:], sin_d[:])

                    def nat_proj(st, wsb0, wsb1):
                        ps = psp.tile([P, H], F32, tag="mm")
                        if st < NT0:
                            xsb, wsb, nd, scol = xt0, wsb0, ND0, st * P
                        else:
                            xsb, wsb, nd, scol = xt1, wsb1, ND1, (st - NT0) * P
                        for dt in range(nd):
                            nc.tensor.matmul(
                                ps[:], xsb[:, dt, scol:scol + P], wsb[:, dt, :],
                                start=(dt == 0), stop=(dt == nd - 1),
                            )
                        return ps

                    def emit_v(st):
                        ps = nat_proj(st, vw0, vw1)
                        nc.vector.tensor_copy(v_sb[:, st, :], ps[:])

                    def emit_knat(vst):
                        ps = psp.tile([P, H], F32, tag="mm", name=f"knp_{vst}")
                        if vst < 3:
                            xsb, wsb, nd, scol = xk0, kw0, ND0, vst * P
                        else:
                            xsb, wsb, nd, scol = xk1, kw1, ND1, 0
                        for dt in range(nd):
                            nc.tensor.matmul(
                                ps[:], xsb[:, dt, scol:scol + P], wsb[:, dt, :],
                                start=(dt == 0), stop=(dt == nd - 1),
                            )
                        k_nat = sp.tile([P, H], BF16, tag="knat")
                        ta = tp.tile([P, HH], F32, tag="rtmp")
                        tb = tp.tile([P, HH], F32, tag="rtmp")
                        tc2_ = tp.tile([P, HH], F32, tag="rtmp")
                        td = tp.tile([P, HH], F32, tag="rtmp")
                        c_ = ckt[:, vst, :]
                        s_ = skt[:, vst, :]
                        x1 = ps[:, 0:HH]
                        x2 = ps[:, HH:H]
                        nc.vector.tensor_tensor(ta[:], x1, c_, MUL)
                        nc.vector.tensor_tensor(tb[:], x2, s_, MUL)
                        nc.vector.tensor_tensor(k_nat[:, 0:HH], ta[:], tb[:], SUB)
                        nc.vector.tensor_tensor(tc2_[:], x2, c_, MUL)
                        nc.vector.tensor_tensor(td[:], x1, s_, MUL)
                        nc.vector.tensor_tensor(k_nat[:, HH:H], tc2_[:], td[:], ADD)
                        nc.gpsimd.dma_start(
                            k_d.rearrange("(a p) h -> p a h", p=P)[:, vst, :], k_nat[:])

                    # modality-1 tiles first: they only need the small
                    # xt1/vw1/kw1 loads, keeping PE busy while xt0 streams
                    for st in (NT0, NT0 + 1):
                        emit_v(st)
                    emit_knat(3)
                    proj_jt(kw0, kw1, kT, chunks=[CHUNKS[2]])
                    # x0 v-tiles: dq-outer accumulation tracks the four
                    # xt0 chunk DMAs so the PE starts before xt0 fully lands
                    vps = {st: psp.tile([P, H], F32, tag="mm", name=f"vps_{st}")
                           for st in range(NT0)}
                    for dq in range(4):
                        for st in range(NT0):
                            for dt in range(dq * 4, dq * 4 + 4):
                                nc.tensor.matmul(
                                    vps[st][:], xt0[:, dt, st * P:(st + 1) * P],
                                    vw0[:, dt, :],
                                    start=(dt == 0), stop=(dt == ND0 - 1),
                                )
                    for st in range(NT0):
                        nc.vector.tensor_copy(v_sb[:, st, :], vps[st][:])
                    nc.gpsimd.dma_start(
                        v_d.rearrange("(a p) h -> p a h", p=P), v_sb[:])

                    # ---- k^T [j, s] + rope (modality-0 chunks) ----
                    proj_jt(kw0, kw1, kT, chunks=CHUNKS[:2])
                    for vst in range(3):
                        emit_knat(vst)

                # kv weights freed -> out-proj weight slices can stream in here
                with (
                    tc.tile_pool(name="outw", bufs=2) as owp,
                    tc.tile_pool(name="ostage", bufs=4) as osp,
                ):
                    # ---- software-pipelined heads ----
                    qTs = {}
                    probs_of = {}
                    recip_of = {}

                    def emit_qproj(h):
                        qw0h = wp.tile([P, ND0, H], BF16, tag="qw0h")
                        qw1h = wp.tile([P, ND1, H], BF16, tag="qw1h")
                        nc.sync.dma_start(qw0h[:], qw0_d[h])
                        nc.sync.dma_start(qw1h[:], qw1_d[h])
                        if h == 0:
                            nc.sync.dma_start(maskT[:], mask_d[:])
                        qT = wp.tile([P, 2, T], BF16, tag="qT", bufs=3)
                        # rope the high-t chunks first: descending-st logits
                        # tiles depend only on the late chunks, so they can
                        # start while the (0,512) chunk is still roping
                        proj_jt(qw0h, qw1h, qT, chunks=list(reversed(CHUNKS)))
                        qTs[h] = qT

                    def logits_steps(h):
                        qT = qTs[h]
                        probs = {}
                        probs_of[h] = probs

                        def step_for(st):
                            def go():
                                if _probw(st) == 1024:
                                    prob_st = probp.tile(
                                        [P, T], BF16, tag="probsF",
                                        bufs=bufs_full, name=f"prF_{h}_{st}")
                                else:
                                    prob_st = probp.tile(
                                        [P, 512], BF16, tag="probsH",
                                        bufs=bufs_half, name=f"prH_{h}_{st}")
                                probs[st] = prob_st
                                lv = live_tc[st]
                                for pi in range(0, len(lv), 2):
                                    grp = lv[pi:pi + 2]
                                    pl = psp.tile([P, 512], F32, tag="mm",
                                                  name=f"pl_{h}_{st}_{pi}")
                                    if len(grp) == 2 and grp[1] == grp[0] + 1:
                                        segs = [(0, grp[0] * TCW, 2 * TCW)]
                                    else:
                                        segs = [(gi, tc4 * TCW, TCW)
                                                for gi, tc4 in enumerate(grp)]
                                    for gi, t0, w in segs:
                                        for jt in range(2):
                                            nc.tensor.matmul(
                                                pl[:, gi * TCW:gi * TCW + w],
                                                kT[:, jt, st * P:(st + 1) * P],
                                                qT[:, jt, t0:t0 + w],
                                                start=(jt == 0), stop=(jt == 1),
                                            )
                                    for gi, tc4 in enumerate(grp):
                                        p0 = pcol[st][tc4]
                                        gl = pl[:, gi * TCW:(gi + 1) * TCW]
                                        nc.scalar.activation(
                                            prob_st[:, p0:p0 + TCW], gl, EXP)
                                        if tile_status[(st, tc4)] == 1:
                                            nc.vector.tensor_tensor(
                                                prob_st[:, p0:p0 + TCW],
                                                prob_st[:, p0:p0 + TCW],
                                                maskT[:, st,
                                                      tc4 * TCW:(tc4 + 1) * TCW],
                                                MUL,
                                            )
                            return go
                        return [step_for(st) for st in reversed(range(NS))
                                if live_tc[st]]

                    def emit_denom(h):
                        probs = probs_of[h]
                        rows = {}
                        for pi in range(0, NTC, 2):
                            pd = psrp.tile([1, 512], F32, tag="denom")
                            for gi, tc4 in enumerate((pi, pi + 1)):
                                na = len(active_st[tc4])
                                for ii, st in enumerate(active_st[tc4]):
                                    p0 = pcol[st][tc4]
                                    nc.tensor.matmul(
                                        pd[:, gi * TCW:(gi + 1) * TCW],
                                        ones_col[:], probs[st][:, p0:p0 + TCW],
                                        start=(ii == 0), stop=(ii == na - 1),
                                    )
                            nc.scalar.activation(pd[:], pd[:], LN)
                            rrow_bf = sp.tile([1, 512], BF16, tag="rrow_bf")
                            nc.scalar.activation(rrow_bf[:], pd[:], EXP, scale=-1.0)
                            rows[pi] = rrow_bf
                        recip_of[h] = rows

                    def emit_bcast(h):
                        rows = recip_of[h]
                        recip_bc = sp.tile([P, T], BF16, tag="recip_bc")
                        for pi in range(0, NTC, 2):
                            t0 = pi * TCW
                            pb = psp.tile([P, 512], F32, tag="mm")
                            nc.tensor.matmul(pb[:], ones_row[:], rows[pi][:])
                            nc.vector.tensor_copy(recip_bc[:, t0:t0 + 512], pb[:])
                        recip_of[h] = recip_bc

                    pv_psum = {}

                    def pv_steps(h):
                        probs = probs_of[h]
                        tiles = {}
                        pv_psum[h] = tiles

                        def step_for(it, tc4):
                            def go():
                                pi = tc4 // 2 * 2
                                if (it, pi) not in tiles:
                                    tiles[(it, pi)] = psp.tile(
                                        [P, 512], F32, tag="mm",
                                        name=f"pe_{h}_{it}_{pi}")
                                pe = tiles[(it, pi)]
                                gi = tc4 % 2
                                na = len(active_st[tc4])
                                for k, st in enumerate(active_st[tc4]):
                                    p0 = pcol[st][tc4]
                                    nc.tensor.matmul(
                                        pe[:, gi * TCW:(gi + 1) * TCW],
                                        v_sb[:, st, it * P:(it + 1) * P],
                                        probs[st][:, p0:p0 + TCW],
                                        start=(k == 0), stop=(k == na - 1),
                                    )
                            return go
                        out = []
                        for it in range(2):
                            for tc4 in range(NTC):
                                if active_st[tc4]:
                                    out.append(step_for(it, tc4))
                        return out

                    def run_steps(a, b):
                        a, b = list(a), list(b)
                        n = max(len(a), len(b))
                        for i in range(n):
                            if i < len(a):
                                a[i]()
                            if i < len(b):
                                b[i]()

                    def emit_pv_norm(h):
                        recip_bc = recip_of[h]
                        tiles = pv_psum[h]
                        for it in range(2):
                            for pi in range(0, NTC, 2):
                                t0 = pi * TCW
                                nc.vector.tensor_tensor(
                                    encn[:, h * 2 + it, t0:t0 + 512],
                                    tiles[(it, pi)][:],
                                    recip_bc[:, t0:t0 + 512], MUL,
                                )
                        del pv_psum[h], recip_of[h]

                    emit_qproj(0)
                    run_steps(logits_steps(0), [])
                    for h in range(1, NH):
                        emit_qproj(h)
                        emit_denom(h - 1)
                        if pipelined:
                            run_steps(logits_steps(h), pv_steps(h - 1))
                        else:
                            run_steps(pv_steps(h - 1), [])
                            run_steps(logits_steps(h), [])
                        emit_bcast(h - 1)
                        emit_pv_norm(h - 1)
                    emit_denom(NH - 1)
                    run_steps([], pv_steps(NH - 1))
                    emit_bcast(NH - 1)
                    emit_pv_norm(NH - 1)

                    # ---- phase C: output projections, dc-outer ----
                    for ow_d, out_d, wdim, ntt, goff in (
                        (ow0_d, out0_d, W0, NT0, 0),
                        (ow1_d, out1_d, W1, NT1, S0),
                    ):
                        for dc in range(wdim // 512):
                            owsl = owp.tile([P, NH * 2, 512], BF16, tag="owsl",
                                            bufs=3)
                            nc.scalar.dma_start(
                                owsl[:], ow_d[:, :, dc * 512:(dc + 1) * 512])
                            for tt in range(ntt):
                                po = psp.tile([P, 512], F32, tag="mm")
                                for hi in range(NH * 2):
                                    nc.tensor.matmul(
                                        po[:],
                                        encn[:, hi, goff + tt * P:goff + (tt + 1) * P],
                                        owsl[:, hi, :],
                                        start=(hi == 0), stop=(hi == NH * 2 - 1),
                                    )
                                stg = osp.tile([P, 512], F32, tag="ostage")
                                nc.vector.tensor_copy(stg[:], po[:])
                                nc.sync.dma_start(
                                    out_d[tt * P:(tt + 1) * P,
                                          dc * 512:(dc + 1) * 512], stg[:])

    nc.finalize()
    return nc


def _swizzle(a, p=P):
    """[n*p, m...] -> [p, n, m...] contiguous (SBUF layout)."""
    n = a.shape[0] // p
    return np.ascontiguousarray(
        a.reshape((n, p) + a.shape[1:]).swapaxes(0, 1)
    )


def _bf16(a):
    return np.asarray(a, dtype=ml_dtypes.bfloat16)


def _prep_core_inputs(inputs):
    x0 = np.asarray(inputs["x0"])
    x1 = np.asarray(inputs["x1"])
    positions = np.asarray(inputs["positions"])
    mask = np.asarray(inputs["attn_mask"])
    q0_w = np.asarray(inputs["q0_w"])
    q1_w = np.asarray(inputs["q1_w"])
    kv0_w = np.asarray(inputs["kv0_w"])
    kv1_w = np.asarray(inputs["kv1_w"])
    out0_w = np.asarray(inputs["out0_w"])
    out1_w = np.asarray(inputs["out1_w"])

    scale = H ** -0.5
    kw0 = _swizzle(_bf16(kv0_w[0, 0]))       # [128, 16, 256]
    vw0 = _swizzle(_bf16(kv0_w[1, 0]))
    kw1 = _swizzle(_bf16(kv1_w[0, 0]))
    vw1 = _swizzle(_bf16(kv1_w[1, 0]))
    qw0 = {}
    qw1 = {}
    ow0 = {}
    ow1 = {}
    for g in range(2):
        hs = NH * g
        qw0[g] = np.stack([_swizzle(_bf16(q0_w[h] * scale)) for h in range(hs, hs + NH)])
        qw1[g] = np.stack([_swizzle(_bf16(q1_w[h] * scale)) for h in range(hs, hs + NH)])
        ow0[g] = _swizzle(_bf16(out0_w[hs:hs + NH].reshape(NH * H, W0)))
        ow1[g] = _swizzle(_bf16(out1_w[hs:hs + NH].reshape(NH * H, W1)))

    in_maps = []
    for c in range(8):
        b, g = c // 2, c % 2
        pos = positions[b].astype(np.float64)            # [T]
        freq_exp = (2.0 / H) * np.arange(HH, dtype=np.float64)
        timescale = MAX_WAVELENGTH ** freq_exp           # [128]
        rad = pos[None, :] / timescale[:, None]          # [128, T]  (j, pos)
        cosjt = _bf16(np.cos(rad))
        sinjt = _bf16(np.sin(rad))
        radT = rad.T                                     # [T, 128]  (pos, j)
        cosT = np.cos(radT).astype(np.float32)           # [T, 128]
        sinT = np.sin(radT).astype(np.float32)
        # k-output work is split across the pair: even core computes
        # s-tiles [0,1,2,6], odd [3,4,5,7] (globals); host re-assembles
        ktiles = [0, 1, 2, 6] if g == 0 else [3, 4, 5, 7]
        xt0_sw = _swizzle(_bf16(x0[b].T))
        xt1_sw = _swizzle(_bf16(x1[b].T))
        xk0 = np.ascontiguousarray(
            xt0_sw[:, :, (ktiles[0]) * P:(ktiles[2] + 1) * P])
        xk1 = np.ascontiguousarray(
            xt1_sw[:, :, (ktiles[3] - NT0) * P:(ktiles[3] - NT0 + 1) * P])
        ckt = _bf16(np.stack([cosT[t * P:(t + 1) * P, :] for t in ktiles], axis=1))
        skt = _bf16(np.stack([sinT[t * P:(t + 1) * P, :] for t in ktiles], axis=1))
        maskT = _swizzle(_bf16(mask[b, 0].T.astype(np.float32)))
        in_maps.append({
            "xt0": xt0_sw,
            "xt1": xt1_sw,
            "xk0": xk0, "xk1": xk1, "ckt": ckt, "skt": skt,
            "qw0": qw0[g], "qw1": qw1[g],
            "kw0": kw0, "kw1": kw1, "vw0": vw0, "vw1": vw1,
            "ow0": ow0[g], "ow1": ow1[g],
            "maskT": maskT,
            "cosjt": cosjt, "sinjt": sinjt,
        })
    return in_maps


def _postprocess(results):
    out0 = np.zeros((B, S0, W0), dtype=np.float32)
    out1 = np.zeros((B, S1, W1), dtype=np.float32)
    k = np.zeros((B, T, 1, H), dtype=np.float32)
    v = np.zeros((B, T, 1, H), dtype=np.float32)
    for b in range(B):
        r0, r1 = results[2 * b], results[2 * b + 1]
        out0[b] = r0["out0p"] + r1["out0p"]
        out1[b] = r0["out1p"] + r1["out1p"]
        for gi, gt in enumerate([0, 1, 2, 6]):
            k[b, gt * P:(gt + 1) * P, 0, :] = r0["k_out"][gi * P:(gi + 1) * P]
        for gi, gt in enumerate([3, 4, 5, 7]):
            k[b, gt * P:(gt + 1) * P, 0, :] = r1["k_out"][gi * P:(gi + 1) * P]
        v[b, :, 0, :] = r0["v_out"]
    idx = np.full((B,), T, dtype=np.int32)
    return out0, out1, idx, k, v


def _mask_tile_status(mask):
    """Per-(s-tile, t-chunk) status over ALL batches: 0=all-false (skip),
    2=all-true (skip mask multiply), 1=mixed."""
    m = np.asarray(mask[:, 0], dtype=bool)           # [B, T, S]
    mT = np.swapaxes(m, 1, 2)                        # [B, S, T]
    status = {}
    for st in range(NS):
        for tc in range(NTC):
            blk = mT[:, st * P:(st + 1) * P, tc * TCW:(tc + 1) * TCW]
            if not blk.any():
                status[(st, tc)] = 0
            elif blk.all():
                status[(st, tc)] = 2
            else:
                status[(st, tc)] = 1
    for tc in range(NTC):  # never leave a t-chunk with no contributing tiles
        if all(status[(st, tc)] == 0 for st in range(NS)):
            for st in range(NS):
                status[(st, tc)] = 1
    return status


_NC_CACHE = {}


def get_nc(tile_status=None):
    key = tuple(sorted(tile_status.items())) if tile_status else None
    if key not in _NC_CACHE:
        _NC_CACHE[key] = build_nc(tile_status)
    return _NC_CACHE[key]


def run(inputs, **kw):
    nc = get_nc(_mask_tile_status(np.asarray(inputs["attn_mask"])))
    in_maps = _prep_core_inputs(inputs)
    try:
        res = run_bass_kernel_spmd(nc, in_maps, core_ids=list(range(8)), **kw)
    except Exception:
        # transient device-state failures have been observed; retry once
        res = run_bass_kernel_spmd(nc, in_maps, core_ids=list(range(8)), **kw)
    return _postprocess(res.results), res


def kernel(**inputs):
    outs, _ = run(inputs)
    return outs


# revision 56
# speedup vs baseline: 1.0125x; 1.0125x over previous
"""Trainium2 Bass kernel for nn_Attention_73203422593618 (sparse_attention).

Two-modality MQA attention layer (B=4, S0=768@W0=2048, S1=256@W1=1024,
T=1024, N=8 query heads, K=1 kv head, H=256, RoPE, causal-masked softmax),
returning (out0, out1, idx, k, v).

Sharding (8 cores): core c -> batch b=c//2 (data parallel), head group
g=c%2 (tensor parallel over 4 of the 8 query heads; MQA k/v replicated
within the pair).  Each core computes partial out0/out1 over its 4 heads;
the pair's two partials are summed on the host at unshard time (the TP
all-reduce).  k/v are produced identically by both cores of a pair; the
host takes the even core's copy.

Device dataflow (per core, all matmuls bf16 with fp32 PSUM accumulation):
  - activations/weights arrive host-swizzled into SBUF layout
    [128-partition, block, free] so every DMA is linear
  - v, k_nat in natural [s, i] layout (stationary = x^T tile), k_nat gets
    RoPE via transposed tables -> k/v DRAM outputs
  - q^T, k^T in [j, t] layout (stationary = weight tile), RoPE via [j, pos]
    tables (q scale H^-0.5 folded into q weights on host)
  - logits^T[s,t] = k^T.T @ q^T per s-tile; probs = exp(logits)*maskT
    (no max subtraction: logits are O(+-6) for this problem's data)
  - denom[1,t] via ones-matmul over s, reciprocal, PE outer-product
    broadcast to [128,t]; normalization folded into enc PSUM->SBUF copy
  - enc^T[i,t] = v.T @ probs^T; out0/out1 = enc_norm^T.T @ ow
"""

import sys

try:
    import concourse  # noqa: F401
except ImportError:  # pragma: no cover
    sys.path.insert(0, "/opt/trn_rl_repo")

import numpy as np
import ml_dtypes

import concourse.bass as bass
import concourse.bacc as bacc
import concourse.mybir as mybir
import concourse.tile as tile
from concourse.bass_utils import run_bass_kernel_spmd

F32 = mybir.dt.float32
BF16 = mybir.dt.bfloat16
MUL = mybir.AluOpType.mult
ADD = mybir.AluOpType.add
SUB = mybir.AluOpType.subtract
EXP = mybir.ActivationFunctionType.Exp
LN = mybir.ActivationFunctionType.Ln
COPY = mybir.ActivationFunctionType.Copy

B, S0, S1 = 4, 768, 256
T = S0 + S1          # 1024
W0, W1 = 2048, 1024
N, H = 8, 256
NH = N // 2          # heads per core (TP=2)
HH = H // 2          # rope half = 128
P = 128
MAX_WAVELENGTH = 10000.0

# position chunks (start, width); starts < S0 are modality 0
CHUNKS = [(0, 512), (512, 256), (768, 256)]
ND0, ND1 = W0 // P, W1 // P    # d-tiles per modality: 16, 8
NS = T // P                    # s-tiles: 8
NT0 = S0 // P                  # t-tiles in modality 0: 6
NT1 = S1 // P                  # t-tiles in modality 1: 2
NTC = 4                        # t-chunks for attention tiles
TCW = T // NTC                 # 256


def _x_slice(xt0, xt1, start, width):
    """Moving-operand source for x^T covering positions [start, start+width)."""
    if start < S0:
        assert start + width <= S0
        return xt0, ND0, start
    return xt1, ND1, start - S0


def build_nc(tile_status=None):
    if tile_status is None:
        tile_status = {(st, tc): 1 for st in range(NS) for tc in range(NTC)}
    live_tc = {st: [tc for tc in range(NTC) if tile_status[(st, tc)]]
               for st in range(NS)}
    pcol = {st: {tc: TCW * i for i, tc in enumerate(live_tc[st])} for st in range(NS)}
    active_st = {tc: [st for st in range(NS) if tile_status[(st, tc)]]
                 for tc in range(NTC)}
    # probs tile widths are quantized to {256, 512, 1024} and share two tags
    def _probw(st):
        n = len(live_tc[st])
        return 512 if n <= 2 else 1024
    n_full = sum(1 for st in range(NS) if _probw(st) == 1024)
    n_half = sum(1 for st in range(NS) if 0 < len(live_tc[st]) and _probw(st) == 512)
    # two heads in flight want 2x buffers; fall back to ~1x when SBUF is tight
    if 2 * (2 * n_full + n_half) > 28:
        bufs_full, bufs_half = n_full + 2, max(n_half + 2, 1)
        pipelined = False   # not enough probs buffers for two heads in flight
    else:
        bufs_full, bufs_half = max(2 * n_full + 1, 1), max(2 * n_half + 2, 1)
        pipelined = True

    nc = bacc.Bacc("TRN2", target_bir_lowering=False, debug=False, num_devices=8)

    xt0_d = nc.declare_dram_parameter("xt0", [P, ND0, S0], BF16, isOutput=False)
    xt1_d = nc.declare_dram_parameter("xt1", [P, ND1, S1], BF16, isOutput=False)
    qw0_d = nc.declare_dram_parameter("qw0", [NH, P, ND0, H], BF16, isOutput=False)
    qw1_d = nc.declare_dram_parameter("qw1", [NH, P, ND1, H], BF16, isOutput=False)
    kw0_d = nc.declare_dram_parameter("kw0", [P, ND0, H], BF16, isOutput=False)
    kw1_d = nc.declare_dram_parameter("kw1", [P, ND1, H], BF16, isOutput=False)
    vw0_d = nc.declare_dram_parameter("vw0", [P, ND0, H], BF16, isOutput=False)
    vw1_d = nc.declare_dram_parameter("vw1", [P, ND1, H], BF16, isOutput=False)
    ow0_d = nc.declare_dram_parameter("ow0", [P, NH * 2, W0], BF16, isOutput=False)
    ow1_d = nc.declare_dram_parameter("ow1", [P, NH * 2, W1], BF16, isOutput=False)
    mask_d = nc.declare_dram_parameter("maskT", [P, NS, T], BF16, isOutput=False)
    cos_d = nc.declare_dram_parameter("cosjt", [P, T], BF16, isOutput=False)
    sin_d = nc.declare_dram_parameter("sinjt", [P, T], BF16, isOutput=False)
    xk0_d = nc.declare_dram_parameter("xk0", [P, ND0, 3 * P], BF16, isOutput=False)
    xk1_d = nc.declare_dram_parameter("xk1", [P, ND1, P], BF16, isOutput=False)
    ckt_d = nc.declare_dram_parameter("ckt", [P, 4, HH], BF16, isOutput=False)
    skt_d = nc.declare_dram_parameter("skt", [P, 4, HH], BF16, isOutput=False)

    out0_d = nc.declare_dram_parameter("out0p", [S0, W0], F32, isOutput=True)
    out1_d = nc.declare_dram_parameter("out1p", [S1, W1], F32, isOutput=True)
    k_d = nc.declare_dram_parameter("k_out", [T // 2, H], F32, isOutput=True)
    v_d = nc.declare_dram_parameter("v_out", [T, H], F32, isOutput=True)

    with tile.TileContext(nc) as tc:
        with (
            tc.tile_pool(name="persist", bufs=1) as pp,
            tc.tile_pool(name="work", bufs=4) as tp,
            tc.tile_pool(name="probs", bufs=8) as probp,
            tc.tile_pool(name="small", bufs=2) as sp,
            tc.tile_pool(name="psum", bufs=7, space="PSUM") as psp,
            tc.tile_pool(name="psum_row", bufs=1, space="PSUM") as psrp,
        ):
            maskT = pp.tile([P, NS, T], BF16, tag="maskT")
            v_sb = pp.tile([P, NS, H], BF16, tag="v_sb")
            kT = pp.tile([P, 2, T], BF16, tag="kT")
            encn = pp.tile([P, NH * 2, T], BF16, tag="encn")
            ones_col = pp.tile([P, 1], BF16, tag="ones_col")
            ones_row = pp.tile([1, P], BF16, tag="ones_row")
            nc.vector.memset(ones_col[:], 1.0)
            nc.vector.memset(ones_row[:], 1.0)
            # HAM warm-up: dummy matmuls with no DMA dependency keep the PE
            # activity monitor busy while the first inputs stream in, so real
            # matmuls start at 2.4 GHz instead of 1.2 GHz
            warm = pp.tile([P, 512], BF16, tag="warm")
            nc.gpsimd.memset(warm[:], 0.0)
            pw = psp.tile([P, 512], F32, tag="mm")
            for i in range(12):
                nc.tensor.matmul(pw[:], warm[:, 0:P], warm[:],
                                 start=(i == 0), stop=(i == 11))

            with (
                tc.tile_pool(name="proj", bufs=1) as pj,
                tc.tile_pool(name="projw", bufs=2) as wp,
            ):
                xt0 = pj.tile([P, ND0, S0], BF16, tag="xt0")
                xt1 = pj.tile([P, ND1, S1], BF16, tag="xt1")
                cosjt = pj.tile([P, T], BF16, tag="cosjt")
                sinjt = pj.tile([P, T], BF16, tag="sinjt")
                xk0 = pj.tile([P, ND0, 3 * P], BF16, tag="xk0")
                xk1 = pj.tile([P, ND1, P], BF16, tag="xk1")
                ckt = pj.tile([P, 4, HH], BF16, tag="ckt")
                skt = pj.tile([P, 4, HH], BF16, tag="skt")

                def rope_jt(pj0, pj1, out_tile, c0, cw):
                    ta = tp.tile([P, 512], BF16, tag="rtmp2", bufs=6)
                    tb = tp.tile([P, 512], BF16, tag="rtmp2", bufs=6)
                    c_ = cosjt[:, c0:c0 + cw]
                    s_ = sinjt[:, c0:c0 + cw]
                    nc.vector.tensor_tensor(ta[:, :cw], pj0[:, :cw], c_, MUL)
                    nc.vector.tensor_tensor(tb[:, :cw], pj1[:, :cw], s_, MUL)
                    nc.vector.tensor_tensor(
                        out_tile[:, 0, c0:c0 + cw], ta[:, :cw], tb[:, :cw], SUB)
                    ta2 = tp.tile([P, 512], BF16, tag="rtmp2", bufs=6)
                    tb2 = tp.tile([P, 512], BF16, tag="rtmp2", bufs=6)
                    nc.vector.tensor_tensor(ta2[:, :cw], pj1[:, :cw], c_, MUL)
                    nc.vector.tensor_tensor(tb2[:, :cw], pj0[:, :cw], s_, MUL)
                    nc.vector.tensor_tensor(
                        out_tile[:, 1, c0:c0 + cw], ta2[:, :cw], tb2[:, :cw], ADD)

                def proj_jt(w0_sb, w1_sb, out_tile, chunks=None):
                    for c0, cw in (chunks or CHUNKS):
                        xsb, nd, off = _x_slice(xt0, xt1, c0, cw)
                        wsb = w0_sb if c0 < S0 else w1_sb
                        pj0 = psp.tile([P, 512], F32, tag="mm")
                        pj1 = psp.tile([P, 512], F32, tag="mm")
                        for jt, pjp in ((0, pj0), (1, pj1)):
                            for dt in range(nd):
                                nc.tensor.matmul(
                                    pjp[:, :cw], wsb[:, dt, jt * P:(jt + 1) * P],
                                    xsb[:, dt, off:off + cw],
                                    start=(dt == 0), stop=(dt == nd - 1),
                                )
                        rope_jt(pj0, pj1, out_tile, c0, cw)

                with tc.tile_pool(name="projkv", bufs=1) as pjw:
                    kw0 = pjw.tile([P, ND0, H], BF16, tag="kw0")
                    kw1 = pjw.tile([P, ND1, H], BF16, tag="kw1")
                    vw0 = pjw.tile([P, ND0, H], BF16, tag="vw0")
                    vw1 = pjw.tile([P, ND1, H], BF16, tag="vw1")

                    nc.sync.dma_start(xt1[:], xt1_d[:])
                    nc.sync.dma_start(vw1[:], vw1_d[:])
                    nc.sync.dma_start(kw1[:], kw1_d[:])
                    nc.sync.dma_start(xk1[:], xk1_d[:])
                    nc.sync.dma_start(ckt[:], ckt_d[:])
                    nc.sync.dma_start(skt[:], skt_d[:])
                    nc.sync.dma_start(vw0[:], vw0_d[:])
                    for dq in range(4):
                        nc.sync.dma_start(xt0[:, dq * 4:(dq + 1) * 4, :],
                                          xt0_d[:, dq * 4:(dq + 1) * 4, :])
                    nc.sync.dma_start(kw0[:], kw0_d[:])
                    nc.sync.dma_start(xk0[:], xk0_d[:])
                    nc.sync.dma_start(cosjt[:], cos_d[:])
                    nc.sync.dma_start(sinjt[:], sin_d[:])

                    def nat_proj(st, wsb0, wsb1):
                        ps = psp.tile([P, H], F32, tag="mm")
                        if st < NT0:
                            xsb, wsb, nd, scol = xt0, wsb0, ND0, st * P
                        else:
                            xsb, wsb, nd, scol = xt1, wsb1, ND1, (st - NT0) * P
                        for dt in range(nd):
                            nc.tensor.matmul(
                                ps[:], xsb[:, dt, scol:scol + P], wsb[:, dt, :],
                                start=(dt == 0), stop=(dt == nd - 1),
                            )
                        return ps

                    def emit_v(st):
                        ps = nat_proj(st, vw0, vw1)
                        nc.vector.tensor_copy(v_sb[:, st, :], ps[:])

                    def emit_knat(vst):
                        ps = psp.tile([P, H], F32, tag="mm", name=f"knp_{vst}")
                        if vst < 3:
                            xsb, wsb, nd, scol = xk0, kw0, ND0, vst * P
                        else:
                            xsb, wsb, nd, scol = xk1, kw1, ND1, 0
                        for dt in range(nd):
                            nc.tensor.matmul(
                                ps[:], xsb[:, dt, scol:scol + P], wsb[:, dt, :],
                                start=(dt == 0), stop=(dt == nd - 1),
                            )
                        k_nat = sp.tile([P, H], BF16, tag="knat")
                        ta = tp.tile([P, HH], F32, tag="rtmp")
                        tb = tp.tile([P, HH], F32, tag="rtmp")
                        tc2_ = tp.tile([P, HH], F32, tag="rtmp")
                        td = tp.tile([P, HH], F32, tag="rtmp")
                        c_ = ckt[:, vst, :]
                        s_ = skt[:, vst, :]
                        x1 = ps[:, 0:HH]
                        x2 = ps[:, HH:H]
                        nc.vector.tensor_tensor(ta[:], x1, c_, MUL)
                        nc.vector.tensor_tensor(tb[:], x2, s_, MUL)
                        nc.vector.tensor_tensor(k_nat[:, 0:HH], ta[:], tb[:], SUB)
                        nc.vector.tensor_tensor(tc2_[:], x2, c_, MUL)
                        nc.vector.tensor_tensor(td[:], x1, s_, MUL)
                        nc.vector.tensor_tensor(k_nat[:, HH:H], tc2_[:], td[:], ADD)
                        nc.gpsimd.dma_start(
                            k_d.rearrange("(a p) h -> p a h", p=P)[:, vst, :], k_nat[:])

                    # modality-1 tiles first: they only need the small
                    # xt1/vw1/kw1 loads, keeping PE busy while xt0 streams
                    for st in (NT0, NT0 + 1):
                        emit_v(st)
                    emit_knat(3)
                    proj_jt(kw0, kw1, kT, chunks=[CHUNKS[2]])
                    # x0 v-tiles: dq-outer accumulation tracks the four
                    # xt0 chunk DMAs so the PE starts before xt0 fully lands
                    vps = {st: psp.tile([P, H], F32, tag="mm", name=f"vps_{st}")
                           for st in range(NT0)}
                    for dq in range(4):
                        for st in range(NT0):
                            for dt in range(dq * 4, dq * 4 + 4):
                                nc.tensor.matmul(
                                    vps[st][:], xt0[:, dt, st * P:(st + 1) * P],
                                    vw0[:, dt, :],
                                    start=(dt == 0), stop=(dt == ND0 - 1),
                                )
                    for st in range(NT0):
                        nc.vector.tensor_copy(v_sb[:, st, :], vps[st][:])
                    nc.gpsimd.dma_start(
                        v_d.rearrange("(a p) h -> p a h", p=P), v_sb[:])

                    # ---- k^T [j, s] + rope (modality-0 chunks) ----
                    proj_jt(kw0, kw1, kT, chunks=CHUNKS[:2])
                    for vst in range(3):
                        emit_knat(vst)

                # kv weights freed -> out-proj weight slices can stream in here
                with (
                    tc.tile_pool(name="outw", bufs=2) as owp,
                    tc.tile_pool(name="ostage", bufs=4) as osp,
                ):
                    # ---- software-pipelined heads ----
                    qTs = {}
                    probs_of = {}
                    recip_of = {}

                    def emit_qproj(h):
                        qw0h = wp.tile([P, ND0, H], BF16, tag="qw0h")
                        qw1h = wp.tile([P, ND1, H], BF16, tag="qw1h")
                        nc.sync.dma_start(qw0h[:], qw0_d[h])
                        nc.sync.dma_start(qw1h[:], qw1_d[h])
                        if h == 0:
                            nc.sync.dma_start(maskT[:], mask_d[:])
                        qT = wp.tile([P, 2, T], BF16, tag="qT", bufs=4)
                        # rope the high-t chunks first: descending-st logits
                        # tiles depend only on the late chunks, so they can
                        # start while the (0,512) chunk is still roping
                        proj_jt(qw0h, qw1h, qT, chunks=list(reversed(CHUNKS)))
                        qTs[h] = qT

                    def logits_steps(h):
                        qT = qTs[h]
                        probs = {}
                        probs_of[h] = probs

                        def step_for(st):
                            def go():
                                if _probw(st) == 1024:
                                    prob_st = probp.tile(
                                        [P, T], BF16, tag="probsF",
                                        bufs=bufs_full, name=f"prF_{h}_{st}")
                                else:
                                    prob_st = probp.tile(
                                        [P, 512], BF16, tag="probsH",
                                        bufs=bufs_half, name=f"prH_{h}_{st}")
                                probs[st] = prob_st
                                lv = live_tc[st]
                                for pi in range(0, len(lv), 2):
                                    grp = lv[pi:pi + 2]
                                    pl = psp.tile([P, 512], F32, tag="mm",
                                                  name=f"pl_{h}_{st}_{pi}")
                                    if len(grp) == 2 and grp[1] == grp[0] + 1:
                                        segs = [(0, grp[0] * TCW, 2 * TCW)]
                                    else:
                                        segs = [(gi, tc4 * TCW, TCW)
                                                for gi, tc4 in enumerate(grp)]
                                    for gi, t0, w in segs:
                                        for jt in range(2):
                                            nc.tensor.matmul(
                                                pl[:, gi * TCW:gi * TCW + w],
                                                kT[:, jt, st * P:(st + 1) * P],
                                                qT[:, jt, t0:t0 + w],
                                                start=(jt == 0), stop=(jt == 1),
                                            )
                                    for gi, tc4 in enumerate(grp):
                                        p0 = pcol[st][tc4]
                                        gl = pl[:, gi * TCW:(gi + 1) * TCW]
                                        nc.scalar.activation(
                                            prob_st[:, p0:p0 + TCW], gl, EXP)
                                        if tile_status[(st, tc4)] == 1:
                                            nc.vector.tensor_tensor(
                                                prob_st[:, p0:p0 + TCW],
                                                prob_st[:, p0:p0 + TCW],
                                                maskT[:, st,
                                                      tc4 * TCW:(tc4 + 1) * TCW],
                                                MUL,
                                            )
                            return go
                        return [step_for(st) for st in reversed(range(NS))
                                if live_tc[st]]

                    def emit_denom(h):
                        probs = probs_of[h]
                        rows = {}
                        for pi in range(0, NTC, 2):
                            pd = psrp.tile([1, 512], F32, tag="denom")
                            for gi, tc4 in enumerate((pi, pi + 1)):
                                na = len(active_st[tc4])
                                for ii, st in enumerate(active_st[tc4]):
                                    p0 = pcol[st][tc4]
                                    nc.tensor.matmul(
                                        pd[:, gi * TCW:(gi + 1) * TCW],
                                        ones_col[:], probs[st][:, p0:p0 + TCW],
                                        start=(ii == 0), stop=(ii == na - 1),
                                    )
                            nc.scalar.activation(pd[:], pd[:], LN)
                            rrow_bf = sp.tile([1, 512], BF16, tag="rrow_bf")
                            nc.scalar.activation(rrow_bf[:], pd[:], EXP, scale=-1.0)
                            rows[pi] = rrow_bf
                        recip_of[h] = rows

                    def emit_bcast(h):
                        rows = recip_of[h]
                        recip_bc = sp.tile([P, T], BF16, tag="recip_bc")
                        for pi in range(0, NTC, 2):
                            t0 = pi * TCW
                            pb = psp.tile([P, 512], F32, tag="mm")
                            nc.tensor.matmul(pb[:], ones_row[:], rows[pi][:])
                            nc.vector.tensor_copy(recip_bc[:, t0:t0 + 512], pb[:])
                        recip_of[h] = recip_bc

                    pv_psum = {}

                    def pv_steps(h):
                        probs = probs_of[h]
                        tiles = {}
                        pv_psum[h] = tiles

                        def step_for(it, tc4):
                            def go():
                                pi = tc4 // 2 * 2
                                if (it, pi) not in tiles:
                                    tiles[(it, pi)] = psp.tile(
                                        [P, 512], F32, tag="mm",
                                        name=f"pe_{h}_{it}_{pi}")
                                pe = tiles[(it, pi)]
                                gi = tc4 % 2
                                na = len(active_st[tc4])
                                for k, st in enumerate(active_st[tc4]):
                                    p0 = pcol[st][tc4]
                                    nc.tensor.matmul(
                                        pe[:, gi * TCW:(gi + 1) * TCW],
                                        v_sb[:, st, it * P:(it + 1) * P],
                                        probs[st][:, p0:p0 + TCW],
                                        start=(k == 0), stop=(k == na - 1),
                                    )
                            return go
                        out = []
                        for it in range(2):
                            for tc4 in range(NTC):
                                if active_st[tc4]:
                                    out.append(step_for(it, tc4))
                        return out

                    def run_steps(a, b):
                        a, b = list(a), list(b)
                        n = max(len(a), len(b))
                        for i in range(n):
                            if i < len(a):
                                a[i]()
                            if i < len(b):
                                b[i]()

                    def emit_pv_norm(h):
                        recip_bc = recip_of[h]
                        tiles = pv_psum[h]
                        for it in range(2):
                            for pi in range(0, NTC, 2):
                                t0 = pi * TCW
                                nc.vector.tensor_tensor(
                                    encn[:, h * 2 + it, t0:t0 + 512],
                                    tiles[(it, pi)][:],
                                    recip_bc[:, t0:t0 + 512], MUL,
                                )
                        del pv_psum[h], recip_of[h]

                    emit_qproj(0)
                    run_steps(logits_steps(0), [])
                    for h in range(1, NH):
                        emit_qproj(h)
                        emit_denom(h - 1)
                        if pipelined:
                            run_steps(logits_steps(h), pv_steps(h - 1))
                        else:
                            run_steps(pv_steps(h - 1), [])
                            run_steps(logits_steps(h), [])
                        emit_bcast(h - 1)
                        emit_pv_norm(h - 1)
                    emit_denom(NH - 1)
                    run_steps([], pv_steps(NH - 1))
                    emit_bcast(NH - 1)
                    emit_pv_norm(NH - 1)

                    # ---- phase C: output projections, dc-outer ----
                    for ow_d, out_d, wdim, ntt, goff in (
                        (ow0_d, out0_d, W0, NT0, 0),
                        (ow1_d, out1_d, W1, NT1, S0),
                    ):
                        for dc in range(wdim // 512):
                            owsl = owp.tile([P, NH * 2, 512], BF16, tag="owsl",
                                            bufs=3)
                            nc.scalar.dma_start(
                                owsl[:], ow_d[:, :, dc * 512:(dc + 1) * 512])
                            for tt in range(ntt):
                                po = psp.tile([P, 512], F32, tag="mm")
                                for hi in range(NH * 2):
                                    nc.tensor.matmul(
                                        po[:],
                                        encn[:, hi, goff + tt * P:goff + (tt + 1) * P],
                                        owsl[:, hi, :],
                                        start=(hi == 0), stop=(hi == NH * 2 - 1),
                                    )
                                stg = osp.tile([P, 512], F32, tag="ostage")
                                nc.vector.tensor_copy(stg[:], po[:])
                                nc.sync.dma_start(
                                    out_d[tt * P:(tt + 1) * P,
                                          dc * 512:(dc + 1) * 512], stg[:])

    nc.finalize()
    return nc


def _swizzle(a, p=P):
    """[n*p, m...] -> [p, n, m...] contiguous (SBUF layout)."""
    n = a.shape[0] // p
    return np.ascontiguousarray(
        a.reshape((n, p) + a.shape[1:]).swapaxes(0, 1)
    )


def _bf16(a):
    return np.asarray(a, dtype=ml_dtypes.bfloat16)


def _prep_core_inputs(inputs):
    x0 = np.asarray(inputs["x0"])
    x1 = np.asarray(inputs["x1"])
    positions = np.asarray(inputs["positions"])
    mask = np.asarray(inputs["attn_mask"])
    q0_w = np.asarray(inputs["q0_w"])
    q1_w = np.asarray(inputs["q1_w"])
    kv0_w = np.asarray(inputs["kv0_w"])
    kv1_w = np.asarray(inputs["kv1_w"])
    out0_w = np.asarray(inputs["out0_w"])
    out1_w = np.asarray(inputs["out1_w"])

    scale = H ** -0.5
    kw0 = _swizzle(_bf16(kv0_w[0, 0]))       # [128, 16, 256]
    vw0 = _swizzle(_bf16(kv0_w[1, 0]))
    kw1 = _swizzle(_bf16(kv1_w[0, 0]))
    vw1 = _swizzle(_bf16(kv1_w[1, 0]))
    qw0 = {}
    qw1 = {}
    ow0 = {}
    ow1 = {}
    for g in range(2):
        hs = NH * g
        qw0[g] = np.stack([_swizzle(_bf16(q0_w[h] * scale)) for h in range(hs, hs + NH)])
        qw1[g] = np.stack([_swizzle(_bf16(q1_w[h] * scale)) for h in range(hs, hs + NH)])
        ow0[g] = _swizzle(_bf16(out0_w[hs:hs + NH].reshape(NH * H, W0)))
        ow1[g] = _swizzle(_bf16(out1_w[hs:hs + NH].reshape(NH * H, W1)))

    in_maps = []
    for c in range(8):
        b, g = c // 2, c % 2
        pos = positions[b].astype(np.float64)            # [T]
        freq_exp = (2.0 / H) * np.arange(HH, dtype=np.float64)
        timescale = MAX_WAVELENGTH ** freq_exp           # [128]
        rad = pos[None, :] / timescale[:, None]          # [128, T]  (j, pos)
        cosjt = _bf16(np.cos(rad))
        sinjt = _bf16(np.sin(rad))
        radT = rad.T                                     # [T, 128]  (pos, j)
        cosT = np.cos(radT).astype(np.float32)           # [T, 128]
        sinT = np.sin(radT).astype(np.float32)
        # k-output work is split across the pair: even core computes
        # s-tiles [0,1,2,6], odd [3,4,5,7] (globals); host re-assembles
        ktiles = [0, 1, 2, 6] if g == 0 else [3, 4, 5, 7]
        xt0_sw = _swizzle(_bf16(x0[b].T))
        xt1_sw = _swizzle(_bf16(x1[b].T))
        xk0 = np.ascontiguousarray(
            xt0_sw[:, :, (ktiles[0]) * P:(ktiles[2] + 1) * P])
        xk1 = np.ascontiguousarray(
            xt1_sw[:, :, (ktiles[3] - NT0) * P:(ktiles[3] - NT0 + 1) * P])
        ckt = _bf16(np.stack([cosT[t * P:(t + 1) * P, :] for t in ktiles], axis=1))
        skt = _bf16(np.stack([sinT[t * P:(t + 1) * P, :] for t in ktiles], axis=1))
        maskT = _swizzle(_bf16(mask[b, 0].T.astype(np.float32)))
        in_maps.append({
            "xt0": xt0_sw,
            "xt1": xt1_sw,
            "xk0": xk0, "xk1": xk1, "ckt": ckt, "skt": skt,
            "qw0": qw0[g], "qw1": qw1[g],
            "kw0": kw0, "kw1": kw1, "vw0": vw0, "vw1": vw1,
            "ow0": ow0[g], "ow1": ow1[g],
            "maskT": maskT,
            "cosjt": cosjt, "sinjt": sinjt,
        })
    return in_maps


def _postprocess(results):
    out0 = np.zeros((B, S0, W0), dtype=np.float32)
    out1 = np.zeros((B, S1, W1), dtype=np.float32)
    k = np.zeros((B, T, 1, H), dtype=np.float32)
    v = np.zeros((B, T, 1, H), dtype=np.float32)
    for b in range(B):
        r0, r1 = results[2 * b], results[2 * b + 1]
        out0[b] = r0["out0p"] + r1["out0p"]
        out1[b] = r0["out1p"] + r1["out1p"]
        for gi, gt in enumerate([0, 1, 2, 6]):
            k[b, gt * P:(gt + 1) * P, 0, :] = r0["k_out"][gi * P:(gi + 1) * P]
        for gi, gt in enumerate([3, 4, 5, 7]):
            k[b, gt * P:(gt + 1) * P, 0, :] = r1["k_out"][gi * P:(gi + 1) * P]
        v[b, :, 0, :] = r0["v_out"]
    idx = np.full((B,), T, dtype=np.int32)
    return out0, out1, idx, k, v


def _mask_tile_status(mask):
    """Per-(s-tile, t-chunk) status over ALL batches: 0=all-false (skip),
    2=all-true (skip mask multiply), 1=mixed."""
    m = np.asarray(mask[:, 0], dtype=bool)           # [B, T, S]
    mT = np.swapaxes(m, 1, 2)                        # [B, S, T]
    status = {}
    for st in range(NS):
        for tc in range(NTC):
            blk = mT[:, st * P:(st + 1) * P, tc * TCW:(tc + 1) * TCW]
            if not blk.any():
                status[(st, tc)] = 0
            elif blk.all():
                status[(st, tc)] = 2
            else:
                status[(st, tc)] = 1
    for tc in range(NTC):  # never leave a t-chunk with no contributing tiles
        if all(status[(st, tc)] == 0 for st in range(NS)):
            for st in range(NS):
                status[(st, tc)] = 1
    return status


_NC_CACHE = {}


def get_nc(tile_status=None):
    key = tuple(sorted(tile_status.items())) if tile_status else None
    if key not in _NC_CACHE:
        _NC_CACHE[key] = build_nc(tile_status)
    return _NC_CACHE[key]


def run(inputs, **kw):
    nc = get_nc(_mask_tile_status(np.asarray(inputs["attn_mask"])))
    in_maps = _prep_core_inputs(inputs)
    try:
        res = run_bass_kernel_spmd(nc, in_maps, core_ids=list(range(8)), **kw)
    except Exception:
        # transient device-state failures have been observed; retry once
        res = run_bass_kernel_spmd(nc, in_maps, core_ids=list(range(8)), **kw)
    return _postprocess(res.results), res


def kernel(**inputs):
    outs, _ = run(inputs)
    return outs


# revision 59
# speedup vs baseline: 1.0144x; 1.0019x over previous
"""Trainium2 Bass kernel for nn_Attention_73203422593618 (sparse_attention).

Two-modality MQA attention layer (B=4, S0=768@W0=2048, S1=256@W1=1024,
T=1024, N=8 query heads, K=1 kv head, H=256, RoPE, causal-masked softmax),
returning (out0, out1, idx, k, v).

Sharding (8 cores): core c -> batch b=c//2 (data parallel), head group
g=c%2 (tensor parallel over 4 of the 8 query heads; MQA k/v replicated
within the pair).  Each core computes partial out0/out1 over its 4 heads;
the pair's two partials are summed on the host at unshard time (the TP
all-reduce).  k/v are produced identically by both cores of a pair; the
host takes the even core's copy.

Device dataflow (per core, all matmuls bf16 with fp32 PSUM accumulation):
  - activations/weights arrive host-swizzled into SBUF layout
    [128-partition, block, free] so every DMA is linear
  - v, k_nat in natural [s, i] layout (stationary = x^T tile), k_nat gets
    RoPE via transposed tables -> k/v DRAM outputs
  - q^T, k^T in [j, t] layout (stationary = weight tile), RoPE via [j, pos]
    tables (q scale H^-0.5 folded into q weights on host)
  - logits^T[s,t] = k^T.T @ q^T per s-tile; probs = exp(logits)*maskT
    (no max subtraction: logits are O(+-6) for this problem's data)
  - denom[1,t] via ones-matmul over s, reciprocal, PE outer-product
    broadcast to [128,t]; normalization folded into enc PSUM->SBUF copy
  - enc^T[i,t] = v.T @ probs^T; out0/out1 = enc_norm^T.T @ ow
"""

import sys

try:
    import concourse  # noqa: F401
except ImportError:  # pragma: no cover
    sys.path.insert(0, "/opt/trn_rl_repo")

import numpy as np
import ml_dtypes

import concourse.bass as bass
import concourse.bacc as bacc
import concourse.mybir as mybir
import concourse.tile as tile
from concourse.bass_utils import run_bass_kernel_spmd

F32 = mybir.dt.float32
BF16 = mybir.dt.bfloat16
MUL = mybir.AluOpType.mult
ADD = mybir.AluOpType.add
SUB = mybir.AluOpType.subtract
EXP = mybir.ActivationFunctionType.Exp
LN = mybir.ActivationFunctionType.Ln
COPY = mybir.ActivationFunctionType.Copy

B, S0, S1 = 4, 768, 256
T = S0 + S1          # 1024
W0, W1 = 2048, 1024
N, H = 8, 256
NH = N // 2          # heads per core (TP=2)
HH = H // 2          # rope half = 128
P = 128
MAX_WAVELENGTH = 10000.0

# position chunks (start, width); starts < S0 are modality 0
CHUNKS = [(0, 512), (512, 256), (768, 256)]
ND0, ND1 = W0 // P, W1 // P    # d-tiles per modality: 16, 8
NS = T // P                    # s-tiles: 8
NT0 = S0 // P                  # t-tiles in modality 0: 6
NT1 = S1 // P                  # t-tiles in modality 1: 2
NTC = 4                        # t-chunks for attention tiles
TCW = T // NTC                 # 256


def _x_slice(xt0, xt1, start, width):
    """Moving-operand source for x^T covering positions [start, start+width)."""
    if start < S0:
        assert start + width <= S0
        return xt0, ND0, start
    return xt1, ND1, start - S0


def build_nc(tile_status=None):
    if tile_status is None:
        tile_status = {(st, tc): 1 for st in range(NS) for tc in range(NTC)}
    live_tc = {st: [tc for tc in range(NTC) if tile_status[(st, tc)]]
               for st in range(NS)}
    pcol = {st: {tc: TCW * i for i, tc in enumerate(live_tc[st])} for st in range(NS)}
    active_st = {tc: [st for st in range(NS) if tile_status[(st, tc)]]
                 for tc in range(NTC)}
    # probs tile widths are quantized to {256, 512, 1024} and share two tags
    def _probw(st):
        n = len(live_tc[st])
        return 512 if n <= 2 else 1024
    n_full = sum(1 for st in range(NS) if _probw(st) == 1024)
    n_half = sum(1 for st in range(NS) if 0 < len(live_tc[st]) and _probw(st) == 512)
    # two heads in flight want 2x buffers; fall back to ~1x when SBUF is tight
    if 2 * (2 * n_full + n_half) > 28:
        bufs_full, bufs_half = n_full + 2, max(n_half + 2, 1)
        pipelined = False   # not enough probs buffers for two heads in flight
    else:
        bufs_full, bufs_half = max(2 * n_full + 1, 1), max(2 * n_half + 2, 1)
        pipelined = True

    nc = bacc.Bacc("TRN2", target_bir_lowering=False, debug=False, num_devices=8)

    xt0_d = nc.declare_dram_parameter("xt0", [P, ND0, S0], BF16, isOutput=False)
    xt1_d = nc.declare_dram_parameter("xt1", [P, ND1, S1], BF16, isOutput=False)
    qw0_d = nc.declare_dram_parameter("qw0", [NH, P, ND0, H], BF16, isOutput=False)
    qw1_d = nc.declare_dram_parameter("qw1", [NH, P, ND1, H], BF16, isOutput=False)
    kw0_d = nc.declare_dram_parameter("kw0", [P, ND0, H], BF16, isOutput=False)
    kw1_d = nc.declare_dram_parameter("kw1", [P, ND1, H], BF16, isOutput=False)
    vw0_d = nc.declare_dram_parameter("vw0", [P, ND0, H], BF16, isOutput=False)
    vw1_d = nc.declare_dram_parameter("vw1", [P, ND1, H], BF16, isOutput=False)
    ow0_d = nc.declare_dram_parameter("ow0", [P, NH * 2, W0], BF16, isOutput=False)
    ow1_d = nc.declare_dram_parameter("ow1", [P, NH * 2, W1], BF16, isOutput=False)
    mask_d = nc.declare_dram_parameter("maskT", [P, NS, T], BF16, isOutput=False)
    cos_d = nc.declare_dram_parameter("cosjt", [P, T], BF16, isOutput=False)
    sin_d = nc.declare_dram_parameter("sinjt", [P, T], BF16, isOutput=False)
    xk0_d = nc.declare_dram_parameter("xk0", [P, ND0, 3 * P], BF16, isOutput=False)
    xk1_d = nc.declare_dram_parameter("xk1", [P, ND1, P], BF16, isOutput=False)
    ckt_d = nc.declare_dram_parameter("ckt", [P, 4, HH], BF16, isOutput=False)
    skt_d = nc.declare_dram_parameter("skt", [P, 4, HH], BF16, isOutput=False)

    out0_d = nc.declare_dram_parameter("out0p", [S0, W0], F32, isOutput=True)
    out1_d = nc.declare_dram_parameter("out1p", [S1, W1], F32, isOutput=True)
    k_d = nc.declare_dram_parameter("k_out", [T // 2, H], F32, isOutput=True)
    v_d = nc.declare_dram_parameter("v_out", [T, H], F32, isOutput=True)

    with tile.TileContext(nc) as tc:
        with (
            tc.tile_pool(name="persist", bufs=1) as pp,
            tc.tile_pool(name="work", bufs=4) as tp,
            tc.tile_pool(name="probs", bufs=8) as probp,
            tc.tile_pool(name="small", bufs=2) as sp,
            tc.tile_pool(name="psum", bufs=7, space="PSUM") as psp,
            tc.tile_pool(name="psum_row", bufs=1, space="PSUM") as psrp,
        ):
            maskT = pp.tile([P, NS, T], BF16, tag="maskT")
            v_sb = pp.tile([P, NS, H], BF16, tag="v_sb")
            kT = pp.tile([P, 2, T], BF16, tag="kT")
            encn = pp.tile([P, NH * 2, T], BF16, tag="encn")
            ones_col = pp.tile([P, 1], BF16, tag="ones_col")
            ones_row = pp.tile([1, P], BF16, tag="ones_row")
            nc.vector.memset(ones_col[:], 1.0)
            nc.vector.memset(ones_row[:], 1.0)
            # HAM warm-up: dummy matmuls with no DMA dependency keep the PE
            # activity monitor busy while the first inputs stream in, so real
            # matmuls start at 2.4 GHz instead of 1.2 GHz
            warm = pp.tile([P, 512], BF16, tag="warm")
            nc.gpsimd.memset(warm[:], 0.0)
            pw = psp.tile([P, 512], F32, tag="mm")
            for i in range(12):
                nc.tensor.matmul(pw[:], warm[:, 0:P], warm[:],
                                 start=(i == 0), stop=(i == 11))

            with (
                tc.tile_pool(name="proj", bufs=1) as pj,
                tc.tile_pool(name="projw", bufs=2) as wp,
            ):
                xt0 = pj.tile([P, ND0, S0], BF16, tag="xt0")
                xt1 = pj.tile([P, ND1, S1], BF16, tag="xt1")
                cosjt = pj.tile([P, T], BF16, tag="cosjt")
                sinjt = pj.tile([P, T], BF16, tag="sinjt")
                xk0 = pj.tile([P, ND0, 3 * P], BF16, tag="xk0")
                xk1 = pj.tile([P, ND1, P], BF16, tag="xk1")
                ckt = pj.tile([P, 4, HH], BF16, tag="ckt")
                skt = pj.tile([P, 4, HH], BF16, tag="skt")

                def rope_jt(pj0, pj1, out_tile, c0, cw):
                    ta = tp.tile([P, 512], BF16, tag="rtmp2", bufs=6)
                    tb = tp.tile([P, 512], BF16, tag="rtmp2", bufs=6)
                    c_ = cosjt[:, c0:c0 + cw]
                    s_ = sinjt[:, c0:c0 + cw]
                    nc.vector.tensor_tensor(ta[:, :cw], pj0[:, :cw], c_, MUL)
                    nc.vector.tensor_tensor(tb[:, :cw], pj1[:, :cw], s_, MUL)
                    nc.vector.tensor_tensor(
                        out_tile[:, 0, c0:c0 + cw], ta[:, :cw], tb[:, :cw], SUB)
                    ta2 = tp.tile([P, 512], BF16, tag="rtmp2", bufs=6)
                    tb2 = tp.tile([P, 512], BF16, tag="rtmp2", bufs=6)
                    nc.vector.tensor_tensor(ta2[:, :cw], pj1[:, :cw], c_, MUL)
                    nc.vector.tensor_tensor(tb2[:, :cw], pj0[:, :cw], s_, MUL)
                    nc.vector.tensor_tensor(
                        out_tile[:, 1, c0:c0 + cw], ta2[:, :cw], tb2[:, :cw], ADD)

                def proj_jt(w0_sb, w1_sb, out_tile, chunks=None):
                    for c0, cw in (chunks or CHUNKS):
                        xsb, nd, off = _x_slice(xt0, xt1, c0, cw)
                        wsb = w0_sb if c0 < S0 else w1_sb
                        pj0 = psp.tile([P, 512], F32, tag="mm")
                        pj1 = psp.tile([P, 512], F32, tag="mm")
                        for jt, pjp in ((0, pj0), (1, pj1)):
                            for dt in range(nd):
                                nc.tensor.matmul(
                                    pjp[:, :cw], wsb[:, dt, jt * P:(jt + 1) * P],
                                    xsb[:, dt, off:off + cw],
                                    start=(dt == 0), stop=(dt == nd - 1),
                                )
                        rope_jt(pj0, pj1, out_tile, c0, cw)

                with tc.tile_pool(name="projkv", bufs=1) as pjw:
                    kw0 = pjw.tile([P, ND0, H], BF16, tag="kw0")
                    kw1 = pjw.tile([P, ND1, H], BF16, tag="kw1")
                    vw0 = pjw.tile([P, ND0, H], BF16, tag="vw0")
                    vw1 = pjw.tile([P, ND1, H], BF16, tag="vw1")

                    nc.sync.dma_start(xt1[:], xt1_d[:])
                    nc.sync.dma_start(vw1[:], vw1_d[:])
                    nc.sync.dma_start(kw1[:], kw1_d[:])
                    nc.sync.dma_start(xk1[:], xk1_d[:])
                    nc.sync.dma_start(ckt[:], ckt_d[:])
                    nc.sync.dma_start(skt[:], skt_d[:])
                    nc.sync.dma_start(vw0[:], vw0_d[:])
                    for dq in range(4):
                        nc.sync.dma_start(xt0[:, dq * 4:(dq + 1) * 4, :],
                                          xt0_d[:, dq * 4:(dq + 1) * 4, :])
                    nc.sync.dma_start(kw0[:], kw0_d[:])
                    nc.sync.dma_start(xk0[:], xk0_d[:])
                    nc.sync.dma_start(cosjt[:], cos_d[:])
                    nc.sync.dma_start(sinjt[:], sin_d[:])

                    def nat_proj(st, wsb0, wsb1):
                        ps = psp.tile([P, H], F32, tag="mm")
                        if st < NT0:
                            xsb, wsb, nd, scol = xt0, wsb0, ND0, st * P
                        else:
                            xsb, wsb, nd, scol = xt1, wsb1, ND1, (st - NT0) * P
                        for dt in range(nd):
                            nc.tensor.matmul(
                                ps[:], xsb[:, dt, scol:scol + P], wsb[:, dt, :],
                                start=(dt == 0), stop=(dt == nd - 1),
                            )
                        return ps

                    def emit_v(st):
                        ps = nat_proj(st, vw0, vw1)
                        nc.vector.tensor_copy(v_sb[:, st, :], ps[:])

                    def emit_knat(vst):
                        ps = psp.tile([P, H], F32, tag="mm", name=f"knp_{vst}")
                        if vst < 3:
                            xsb, wsb, nd, scol = xk0, kw0, ND0, vst * P
                        else:
                            xsb, wsb, nd, scol = xk1, kw1, ND1, 0
                        for dt in range(nd):
                            nc.tensor.matmul(
                                ps[:], xsb[:, dt, scol:scol + P], wsb[:, dt, :],
                                start=(dt == 0), stop=(dt == nd - 1),
                            )
                        k_nat = sp.tile([P, H], BF16, tag="knat")
                        ta = tp.tile([P, HH], F32, tag="rtmp")
                        tb = tp.tile([P, HH], F32, tag="rtmp")
                        tc2_ = tp.tile([P, HH], F32, tag="rtmp")
                        td = tp.tile([P, HH], F32, tag="rtmp")
                        c_ = ckt[:, vst, :]
                        s_ = skt[:, vst, :]
                        x1 = ps[:, 0:HH]
                        x2 = ps[:, HH:H]
                        nc.vector.tensor_tensor(ta[:], x1, c_, MUL)
                        nc.vector.tensor_tensor(tb[:], x2, s_, MUL)
                        nc.vector.tensor_tensor(k_nat[:, 0:HH], ta[:], tb[:], SUB)
                        nc.vector.tensor_tensor(tc2_[:], x2, c_, MUL)
                        nc.vector.tensor_tensor(td[:], x1, s_, MUL)
                        nc.vector.tensor_tensor(k_nat[:, HH:H], tc2_[:], td[:], ADD)
                        nc.gpsimd.dma_start(
                            k_d.rearrange("(a p) h -> p a h", p=P)[:, vst, :], k_nat[:])

                    # modality-1 tiles first: they only need the small
                    # xt1/vw1/kw1 loads, keeping PE busy while xt0 streams
                    for st in (NT0, NT0 + 1):
                        emit_v(st)
                    emit_knat(3)
                    proj_jt(kw0, kw1, kT, chunks=[CHUNKS[2]])
                    # x0 v-tiles: dq-outer accumulation tracks the four
                    # xt0 chunk DMAs so the PE starts before xt0 fully lands
                    vps = {st: psp.tile([P, H], F32, tag="mm", name=f"vps_{st}")
                           for st in range(NT0)}
                    for dq in range(4):
                        for st in range(NT0):
                            for dt in range(dq * 4, dq * 4 + 4):
                                nc.tensor.matmul(
                                    vps[st][:], xt0[:, dt, st * P:(st + 1) * P],
                                    vw0[:, dt, :],
                                    start=(dt == 0), stop=(dt == ND0 - 1),
                                )
                    for st in range(NT0):
                        nc.vector.tensor_copy(v_sb[:, st, :], vps[st][:])
                    nc.gpsimd.dma_start(
                        v_d.rearrange("(a p) h -> p a h", p=P), v_sb[:])

                    # ---- k^T [j, s] + rope (modality-0 chunks) ----
                    proj_jt(kw0, kw1, kT, chunks=CHUNKS[:2])
                    for vst in range(3):
                        emit_knat(vst)

                # kv weights freed -> out-proj weight slices can stream in here
                with (
                    tc.tile_pool(name="outw", bufs=2) as owp,
                    tc.tile_pool(name="ostage", bufs=4) as osp,
                ):
                    # ---- software-pipelined heads ----
                    qTs = {}
                    probs_of = {}
                    recip_of = {}

                    def emit_qproj(h):
                        qw0h = wp.tile([P, ND0, H], BF16, tag="qw0h")
                        qw1h = wp.tile([P, ND1, H], BF16, tag="qw1h")
                        nc.sync.dma_start(qw0h[:], qw0_d[h])
                        nc.sync.dma_start(qw1h[:], qw1_d[h])
                        if h == 0:
                            nc.sync.dma_start(maskT[:], mask_d[:])
                        qT = wp.tile([P, 2, T], BF16, tag="qT", bufs=4)
                        # rope the high-t chunks first: descending-st logits
                        # tiles depend only on the late chunks, so they can
                        # start while the (0,512) chunk is still roping
                        proj_jt(qw0h, qw1h, qT, chunks=list(reversed(CHUNKS)))
                        qTs[h] = qT

                    def logits_steps(h):
                        qT = qTs[h]
                        probs = {}
                        probs_of[h] = probs

                        def step_for(st):
                            def go():
                                if _probw(st) == 1024:
                                    prob_st = probp.tile(
                                        [P, T], BF16, tag="probsF",
                                        bufs=bufs_full, name=f"prF_{h}_{st}")
                                else:
                                    prob_st = probp.tile(
                                        [P, 512], BF16, tag="probsH",
                                        bufs=bufs_half, name=f"prH_{h}_{st}")
                                probs[st] = prob_st
                                lv = live_tc[st]
                                for pi in range(0, len(lv), 2):
                                    grp = lv[pi:pi + 2]
                                    pl = psp.tile([P, 512], F32, tag="mm",
                                                  name=f"pl_{h}_{st}_{pi}")
                                    if len(grp) == 2 and grp[1] == grp[0] + 1:
                                        segs = [(0, grp[0] * TCW, 2 * TCW)]
                                    else:
                                        segs = [(gi, tc4 * TCW, TCW)
                                                for gi, tc4 in enumerate(grp)]
                                    for gi, t0, w in segs:
                                        for jt in range(2):
                                            nc.tensor.matmul(
                                                pl[:, gi * TCW:gi * TCW + w],
                                                kT[:, jt, st * P:(st + 1) * P],
                                                qT[:, jt, t0:t0 + w],
                                                start=(jt == 0), stop=(jt == 1),
                                            )
                                    for gi, tc4 in enumerate(grp):
                                        p0 = pcol[st][tc4]
                                        gl = pl[:, gi * TCW:(gi + 1) * TCW]
                                        nc.scalar.activation(
                                            prob_st[:, p0:p0 + TCW], gl, EXP)
                                        if tile_status[(st, tc4)] == 1:
                                            nc.vector.tensor_tensor(
                                                prob_st[:, p0:p0 + TCW],
                                                prob_st[:, p0:p0 + TCW],
                                                maskT[:, st,
                                                      tc4 * TCW:(tc4 + 1) * TCW],
                                                MUL,
                                            )
                            return go
                        return [step_for(st) for st in reversed(range(NS))
                                if live_tc[st]]

                    def emit_denom(h):
                        probs = probs_of[h]
                        rows = {}
                        for pi in range(0, NTC, 2):
                            pd = psrp.tile([1, 512], F32, tag="denom")
                            for gi, tc4 in enumerate((pi, pi + 1)):
                                na = len(active_st[tc4])
                                for ii, st in enumerate(active_st[tc4]):
                                    p0 = pcol[st][tc4]
                                    nc.tensor.matmul(
                                        pd[:, gi * TCW:(gi + 1) * TCW],
                                        ones_col[:], probs[st][:, p0:p0 + TCW],
                                        start=(ii == 0), stop=(ii == na - 1),
                                    )
                            nc.scalar.activation(pd[:], pd[:], LN)
                            rrow_bf = sp.tile([1, 512], BF16, tag="rrow_bf")
                            nc.scalar.activation(rrow_bf[:], pd[:], EXP, scale=-1.0)
                            rows[pi] = rrow_bf
                        recip_of[h] = rows

                    def emit_bcast(h):
                        rows = recip_of[h]
                        recip_bc = sp.tile([P, T], BF16, tag="recip_bc")
                        for pi in range(0, NTC, 2):
                            t0 = pi * TCW
                            pb = psp.tile([P, 512], F32, tag="mm")
                            nc.tensor.matmul(pb[:], ones_row[:], rows[pi][:])
                            nc.vector.tensor_copy(recip_bc[:, t0:t0 + 512], pb[:])
                        recip_of[h] = recip_bc

                    pv_psum = {}

                    def pv_steps(h):
                        probs = probs_of[h]
                        tiles = {}
                        pv_psum[h] = tiles

                        def step_for(it, tc4):
                            def go():
                                pi = tc4 // 2 * 2
                                if (it, pi) not in tiles:
                                    tiles[(it, pi)] = psp.tile(
                                        [P, 512], F32, tag="mm",
                                        name=f"pe_{h}_{it}_{pi}")
                                pe = tiles[(it, pi)]
                                gi = tc4 % 2
                                na = len(active_st[tc4])
                                for k, st in enumerate(active_st[tc4]):
                                    p0 = pcol[st][tc4]
                                    nc.tensor.matmul(
                                        pe[:, gi * TCW:(gi + 1) * TCW],
                                        v_sb[:, st, it * P:(it + 1) * P],
                                        probs[st][:, p0:p0 + TCW],
                                        start=(k == 0), stop=(k == na - 1),
                                    )
                            return go
                        out = []
                        for it in range(2):
                            for tc4 in range(NTC):
                                if active_st[tc4]:
                                    out.append(step_for(it, tc4))
                        return out

                    def run_steps(a, b):
                        a, b = list(a), list(b)
                        n = max(len(a), len(b))
                        for i in range(n):
                            if i < len(a):
                                a[i]()
                            if i < len(b):
                                b[i]()

                    def emit_pv_norm(h):
                        recip_bc = recip_of[h]
                        tiles = pv_psum[h]
                        for it in range(2):
                            for pi in range(0, NTC, 2):
                                t0 = pi * TCW
                                nc.vector.tensor_tensor(
                                    encn[:, h * 2 + it, t0:t0 + 512],
                                    tiles[(it, pi)][:],
                                    recip_bc[:, t0:t0 + 512], MUL,
                                )
                        del pv_psum[h], recip_of[h]

                    emit_qproj(0)
                    run_steps(logits_steps(0), [])
                    for h in range(1, NH):
                        emit_qproj(h)
                        emit_denom(h - 1)
                        if pipelined:
                            run_steps(logits_steps(h), pv_steps(h - 1))
                        else:
                            run_steps(pv_steps(h - 1), [])
                            run_steps(logits_steps(h), [])
                        emit_bcast(h - 1)
                        emit_pv_norm(h - 1)
                    emit_denom(NH - 1)
                    run_steps([], pv_steps(NH - 1))
                    emit_bcast(NH - 1)
                    emit_pv_norm(NH - 1)

                    # ---- phase C: output projections, dc-outer ----
                    for ow_d, out_d, wdim, ntt, goff in (
                        (ow0_d, out0_d, W0, NT0, 0),
                        (ow1_d, out1_d, W1, NT1, S0),
                    ):
                        for dc in range(wdim // 512):
                            owsl = owp.tile([P, NH * 2, 512], BF16, tag="owsl",
                                            bufs=3)
                            nc.scalar.dma_start(
                                owsl[:], ow_d[:, :, dc * 512:(dc + 1) * 512])
                            for tt in range(ntt):
                                po = psp.tile([P, 512], F32, tag="mm")
                                for hi in range(NH * 2):
                                    nc.tensor.matmul(
                                        po[:],
                                        encn[:, hi, goff + tt * P:goff + (tt + 1) * P],
                                        owsl[:, hi, :],
                                        start=(hi == 0), stop=(hi == NH * 2 - 1),
                                    )
                                stg = osp.tile([P, 512], F32, tag="ostage")
                                nc.vector.tensor_copy(stg[:], po[:])
                                nc.sync.dma_start(
                                    out_d[tt * P:(tt + 1) * P,
                                          dc * 512:(dc + 1) * 512], stg[:])

    nc.finalize()
    return nc


def _swizzle(a, p=P):
    """[n*p, m...] -> [p, n, m...] contiguous (SBUF layout)."""
    n = a.shape[0] // p
    return np.ascontiguousarray(
        a.reshape((n, p) + a.shape[1:]).swapaxes(0, 1)
    )


def _bf16(a):
    return np.asarray(a, dtype=ml_dtypes.bfloat16)


def _prep_core_inputs(inputs):
    x0 = np.asarray(inputs["x0"])
    x1 = np.asarray(inputs["x1"])
    positions = np.asarray(inputs["positions"])
    mask = np.asarray(inputs["attn_mask"])
    q0_w = np.asarray(inputs["q0_w"])
    q1_w = np.asarray(inputs["q1_w"])
    kv0_w = np.asarray(inputs["kv0_w"])
    kv1_w = np.asarray(inputs["kv1_w"])
    out0_w = np.asarray(inputs["out0_w"])
    out1_w = np.asarray(inputs["out1_w"])

    scale = H ** -0.5
    kw0 = _swizzle(_bf16(kv0_w[0, 0]))       # [128, 16, 256]
    vw0 = _swizzle(_bf16(kv0_w[1, 0]))
    kw1 = _swizzle(_bf16(kv1_w[0, 0]))
    vw1 = _swizzle(_bf16(kv1_w[1, 0]))
    qw0 = {}
    qw1 = {}
    ow0 = {}
    ow1 = {}
    for g in range(2):
        hs = NH * g
        qw0[g] = np.stack([_swizzle(_bf16(q0_w[h] * scale)) for h in range(hs, hs + NH)])
        qw1[g] = np.stack([_swizzle(_bf16(q1_w[h] * scale)) for h in range(hs, hs + NH)])
        ow0[g] = _swizzle(_bf16(out0_w[hs:hs + NH].reshape(NH * H, W0)))
        ow1[g] = _swizzle(_bf16(out1_w[hs:hs + NH].reshape(NH * H, W1)))

    in_maps = []
    for c in range(8):
        b, g = c // 2, c % 2
        pos = positions[b].astype(np.float64)            # [T]
        freq_exp = (2.0 / H) * np.arange(HH, dtype=np.float64)
        timescale = MAX_WAVELENGTH ** freq_exp           # [128]
        rad = pos[None, :] / timescale[:, None]          # [128, T]  (j, pos)
        cosjt = _bf16(np.cos(rad))
        sinjt = _bf16(np.sin(rad))
        radT = rad.T                                     # [T, 128]  (pos, j)
        cosT = np.cos(radT).astype(np.float32)           # [T, 128]
        sinT = np.sin(radT).astype(np.float32)
        # k-output work is split across the pair: even core computes
        # s-tiles [0,1,2,6], odd [3,4,5,7] (globals); host re-assembles
        ktiles = [0, 1, 2, 6] if g == 0 else [3, 4, 5, 7]
        xt0_sw = _swizzle(_bf16(x0[b].T))
        xt1_sw = _swizzle(_bf16(x1[b].T))
        xk0 = np.ascontiguousarray(
            xt0_sw[:, :, (ktiles[0]) * P:(ktiles[2] + 1) * P])
        xk1 = np.ascontiguousarray(
            xt1_sw[:, :, (ktiles[3] - NT0) * P:(ktiles[3] - NT0 + 1) * P])
        ckt = _bf16(np.stack([cosT[t * P:(t + 1) * P, :] for t in ktiles], axis=1))
        skt = _bf16(np.stack([sinT[t * P:(t + 1) * P, :] for t in ktiles], axis=1))
        maskT = _swizzle(_bf16(mask[b, 0].T.astype(np.float32)))
        in_maps.append({
            "xt0": xt0_sw,
            "xt1": xt1_sw,
            "xk0": xk0, "xk1": xk1, "ckt": ckt, "skt": skt,
            "qw0": qw0[g], "qw1": qw1[g],
            "kw0": kw0, "kw1": kw1, "vw0": vw0, "vw1": vw1,
            "ow0": ow0[g], "ow1": ow1[g],
            "maskT": maskT,
            "cosjt": cosjt, "sinjt": sinjt,
        })
    return in_maps


def _postprocess(results):
    out0 = np.zeros((B, S0, W0), dtype=np.float32)
    out1 = np.zeros((B, S1, W1), dtype=np.float32)
    k = np.zeros((B, T, 1, H), dtype=np.float32)
    v = np.zeros((B, T, 1, H), dtype=np.float32)
    for b in range(B):
        r0, r1 = results[2 * b], results[2 * b + 1]
        out0[b] = r0["out0p"] + r1["out0p"]
        out1[b] = r0["out1p"] + r1["out1p"]
        for gi, gt in enumerate([0, 1, 2, 6]):
            k[b, gt * P:(gt + 1) * P, 0, :] = r0["k_out"][gi * P:(gi + 1) * P]
        for gi, gt in enumerate([3, 4, 5, 7]):
            k[b, gt * P:(gt + 1) * P, 0, :] = r1["k_out"][gi * P:(gi + 1) * P]
        v[b, :, 0, :] = r0["v_out"]
    idx = np.full((B,), T, dtype=np.int32)
    return out0, out1, idx, k, v


def _mask_tile_status(mask):
    """Per-(s-tile, t-chunk) status over ALL batches: 0=all-false (skip),
    2=all-true (skip mask multiply), 1=mixed."""
    m = np.asarray(mask[:, 0], dtype=bool)           # [B, T, S]
    mT = np.swapaxes(m, 1, 2)                        # [B, S, T]
    status = {}
    for st in range(NS):
        for tc in range(NTC):
            blk = mT[:, st * P:(st + 1) * P, tc * TCW:(tc + 1) * TCW]
            if not blk.any():
                status[(st, tc)] = 0
            elif blk.all():
                status[(st, tc)] = 2
            else:
                status[(st, tc)] = 1
    for tc in range(NTC):  # never leave a t-chunk with no contributing tiles
        if all(status[(st, tc)] == 0 for st in range(NS)):
            for st in range(NS):
                status[(st, tc)] = 1
    return status


_NC_CACHE = {}


def get_nc(tile_status=None):
    key = tuple(sorted(tile_status.items())) if tile_status else None
    if key not in _NC_CACHE:
        _NC_CACHE[key] = build_nc(tile_status)
    return _NC_CACHE[key]


def run(inputs, **kw):
    nc = get_nc(_mask_tile_status(np.asarray(inputs["attn_mask"])))
    in_maps = _prep_core_inputs(inputs)
    try:
        res = run_bass_kernel_spmd(nc, in_maps, core_ids=list(range(8)), **kw)
    except Exception:
        # transient device-state failures have been observed; retry once
        res = run_bass_kernel_spmd(nc, in_maps, core_ids=list(range(8)), **kw)
    return _postprocess(res.results), res


def kernel(**inputs):
    outs, _ = run(inputs)
    return outs
